# revision 2
# baseline (speedup 1.0000x reference)
"""Batched sparse-dense matmul (COO SpMM) on 8 Trainium2 NeuronCores.

Problem: y[b, r] = sum_k vals[k] * x[b, cols[k]] where rows[k] == r.
  x: [128, 16384] f32, vals/rows/cols: [524288], y: [128, 8192] f32.

Strategy (row-sharded, output-stationary):
  - Host: sort nonzeros by row; shard the 8192 rows across the 8 cores
    (1024 rows each).  Within a core, sort rows by nnz (descending) and
    group them into 8 bins of 128 rows; pad every row of bin j to the
    bin's max nnz K_j (rounded to the gather chunk size).  This gives a
    fully regular ELL layout with only a few % padding.  Partition p of
    the SBUF holds the 8 rows ranked (j*128+p) (one per bin).
  - Device: for each chunk of CS slots, dma_gather pulls the needed
    rows of x^T (one 512B row of x^T per nonzero) from HBM into SBUF in
    the wrapped layout [128 part, CS, 128 batch]; DVE multiplies by the
    per-slot vals (broadcast along the batch axis) and reduces each
    chunk over its slots into a partials buffer; a final pass reduces
    each bin's partials into y^T [128 part, 8 bins, 128 batch].
  - Host: scatter the per-core y^T tiles back into y[b, r] using the
    recorded row permutation.
"""

import sys

sys.path.insert(0, "/opt/trn_rl_repo")

import numpy as np

import concourse.bacc as bacc
import concourse.bass as bass
import concourse.mybir as mybir
import concourse.tile as tile
from concourse.bass_utils import run_bass_kernel_spmd

B = 128        # batch
R = 8192       # rows of sparse matrix / output features
C = 16384      # cols of sparse matrix / input features
NNZ = 524288
NCORES = 8
RC = R // NCORES      # rows per core
NBINS = RC // 128     # row-bins per core (rows per partition)
CS = 16               # ELL slots per gather chunk (num_idxs = CS*128)


def _prep(x, vals, rows, cols):
    """Host-side preprocessing: ELL layout construction + index perm."""
    order = np.argsort(rows, kind="stable")
    r_s = rows[order]
    c_s = cols[order].astype(np.int64)
    v_s = vals[order]

    counts = np.bincount(rows, minlength=R).astype(np.int64)
    row_start = np.zeros(R + 1, dtype=np.int64)
    np.cumsum(counts, out=row_start[1:])

    # rank rows within each core by nnz desc
    rank = np.empty((NCORES, RC), dtype=np.int64)
    for m in range(NCORES):
        cm = counts[m * RC:(m + 1) * RC]
        rank[m] = np.argsort(-cm, kind="stable")

    # common (across cores) bin widths K_j, padded to CS
    K = np.zeros(NBINS, dtype=np.int64)
    for j in range(NBINS):
        for m in range(NCORES):
            kj = counts[m * RC + rank[m, j * 128]]
            K[j] = max(K[j], kj)
    K = np.maximum(((K + CS - 1) // CS) * CS, CS)
    off = np.zeros(NBINS + 1, dtype=np.int64)
    np.cumsum(K, out=off[1:])
    S = int(off[-1])

    cols_ell = np.zeros((NCORES, 128, S), dtype=np.int64)
    vals_ell = np.zeros((NCORES, 128, S), dtype=np.float32)
    rid = np.empty((NCORES, NBINS, 128), dtype=np.int64)
    for m in range(NCORES):
        for j in range(NBINS):
            rows_bin = m * RC + rank[m, j * 128:(j + 1) * 128]   # [128]
            rid[m, j] = rows_bin
            kj = int(K[j])
            cnt = counts[rows_bin]                               # [128]
            t = np.arange(kj)[None, :]                           # [1, kj]
            valid = t < cnt[:, None]                             # [128, kj]
            src = row_start[rows_bin][:, None] + np.minimum(t, np.maximum(cnt[:, None] - 1, 0))
            seg = slice(int(off[j]), int(off[j]) + kj)
            cols_ell[m, :, seg] = np.where(valid, c_s[src], 0)
            vals_ell[m, :, seg] = np.where(valid, v_s[src], 0.0)

    # dma_gather linear order: element i -> partition i%128, slot i//128
    # we want partition p, slot s  <=>  i = s*128 + p
    idx_tiles = []
    val_tiles = []
    for m in range(NCORES):
        idx_lin = np.ascontiguousarray(cols_ell[m].T).reshape(-1)   # [S*128]
        idx_wr = np.ascontiguousarray(idx_lin.reshape(S * 8, 16).T) # [16, S*8]
        idx_tiles.append(np.ascontiguousarray(np.tile(idx_wr, (8, 1))).astype(np.int16))
        val_tiles.append(np.ascontiguousarray(vals_ell[m]))

    return idx_tiles, val_tiles, rid, K, off, S


def _build_nc(S, K, off):
    nsub = S // CS
    nc = bacc.Bacc("TRN2", target_bir_lowering=False, debug=False)
    xt_d = nc.dram_tensor("xt", [C, B], mybir.dt.float32, kind="ExternalInput")
    idx_d = nc.dram_tensor("idx", [128, S * 8], mybir.dt.int16, kind="ExternalInput")
    val_d = nc.dram_tensor("vals", [128, S], mybir.dt.float32, kind="ExternalInput")
    y_d = nc.dram_tensor("y", [128, NBINS * B], mybir.dt.float32, kind="ExternalOutput")

    with tile.TileContext(nc) as tc:
        with (
            tc.tile_pool(name="const", bufs=1) as cpool,
            tc.tile_pool(name="g", bufs=3) as gpool,
            tc.tile_pool(name="gs", bufs=3) as gspool,
        ):
            idx_t = cpool.tile([128, S * 8], mybir.dt.int16)
            nc.sync.dma_start(out=idx_t[:], in_=idx_d[:])
            val_t = cpool.tile([128, S], mybir.dt.float32)
            nc.sync.dma_start(out=val_t[:], in_=val_d[:])
            p_t = cpool.tile([128, nsub, B], mybir.dt.float32)
            y_t = cpool.tile([128, NBINS, B], mybir.dt.float32)

            for u in range(nsub):
                g = gpool.tile([128, CS, B], mybir.dt.float32)
                nc.gpsimd.dma_gather(
                    g[:],
                    xt_d[:],
                    idx_t[:, u * CS * 8:(u + 1) * CS * 8],
                    CS * 128,
                    CS * 128,
                    B,
                    single_packet=False,
                )
                gs = gspool.tile([128, CS, B], mybir.dt.float32)
                vb = val_t[:, u * CS:(u + 1) * CS].unsqueeze(2).to_broadcast([128, CS, B])
                nc.vector.tensor_tensor(
                    out=gs[:], in0=g[:], in1=vb, op=mybir.AluOpType.mult
                )
                nc.vector.tensor_reduce(
                    out=p_t[:, u, :],
                    in_=gs[:].transpose([0, 2, 1]),
                    axis=mybir.AxisListType.X,
                    op=mybir.AluOpType.add,
                )
            for j in range(NBINS):
                q0 = int(off[j]) // CS
                qj = int(K[j]) // CS
                nc.vector.tensor_reduce(
                    out=y_t[:, j, :],
                    in_=p_t[:, q0:q0 + qj, :].transpose([0, 2, 1]),
                    axis=mybir.AxisListType.X,
                    op=mybir.AluOpType.add,
                )
            nc.sync.dma_start(out=y_d[:], in_=y_t[:])
    nc.compile()
    return nc


_CACHE = {}


def _get_nc(S, K, off):
    key = (S, tuple(int(k) for k in K))
    if key not in _CACHE:
        _CACHE[key] = _build_nc(S, K, off)
    return _CACHE[key]


def kernel(x_batched, M_vals, M_row_idx, M_col_idx, _want_results=False, **_):
    x = np.asarray(x_batched, dtype=np.float32)
    vals = np.asarray(M_vals, dtype=np.float32)
    rows = np.asarray(M_row_idx, dtype=np.int64)
    cols = np.asarray(M_col_idx, dtype=np.int64)

    idx_tiles, val_tiles, rid, K, off, S = _prep(x, vals, rows, cols)
    xt = np.ascontiguousarray(x.T)  # [C, B]

    nc = _get_nc(S, K, off)
    in_maps = [
        {"xt": xt, "idx": idx_tiles[m], "vals": val_tiles[m]}
        for m in range(NCORES)
    ]
    res = run_bass_kernel_spmd(nc, in_maps, core_ids=list(range(NCORES)))

    y = np.zeros((B, R), dtype=np.float32)
    for m in range(NCORES):
        ydev = res.results[m]["y"].reshape(128, NBINS, B)
        yt = ydev.transpose(1, 0, 2).reshape(NBINS * 128, B)
        y[:, rid[m].reshape(-1)] = yt.T
    if _want_results:
        return y, res
    return y


# revision 4
# speedup vs baseline: 1.4203x; 1.4203x over previous
"""Batched sparse-dense matmul (COO SpMM) on 8 Trainium2 NeuronCores.

Problem: y[b, r] = sum_k vals[k] * x[b, cols[k]] where rows[k] == r.
  x: [128, 16384] f32, vals/rows/cols: [524288], y: [128, 8192] f32.

Strategy (row-sharded, output-stationary):
  - Host: sort nonzeros by row; shard the 8192 rows across the 8 cores
    (1024 rows each).  Within a core, sort rows by nnz (descending) and
    group them into 8 bins of 128 rows; pad every row of bin j to the
    bin's max nnz K_j (rounded to the gather chunk size).  This gives a
    fully regular ELL layout with only a few % padding.  Partition p of
    the SBUF holds the 8 rows ranked (j*128+p) (one per bin).
  - Device: for each chunk of CS slots, dma_gather pulls the needed
    rows of x^T (one 512B row of x^T per nonzero) from HBM into SBUF in
    the wrapped layout [128 part, CS, 128 batch]; DVE multiplies by the
    per-slot vals (broadcast along the batch axis) and reduces each
    chunk over its slots into a partials buffer; a final pass reduces
    each bin's partials into y^T [128 part, 8 bins, 128 batch].
  - Host: scatter the per-core y^T tiles back into y[b, r] using the
    recorded row permutation.
"""

import sys

sys.path.insert(0, "/opt/trn_rl_repo")

import numpy as np

import concourse.bacc as bacc
import concourse.bass as bass
import concourse.mybir as mybir
import concourse.tile as tile
from concourse.bass_utils import run_bass_kernel_spmd

B = 128        # batch
R = 8192       # rows of sparse matrix / output features
C = 16384      # cols of sparse matrix / input features
NNZ = 524288
NCORES = 8
RC = R // NCORES      # rows per core
NBINS = RC // 128     # row-bins per core (rows per partition)
CS = 16               # ELL slots per gather chunk (num_idxs = CS*128)


def _prep(x, vals, rows, cols):
    """Host-side preprocessing: ELL layout construction + index perm."""
    order = np.argsort(rows, kind="stable")
    r_s = rows[order]
    c_s = cols[order].astype(np.int64)
    v_s = vals[order]

    counts = np.bincount(rows, minlength=R).astype(np.int64)
    row_start = np.zeros(R + 1, dtype=np.int64)
    np.cumsum(counts, out=row_start[1:])

    # rank rows within each core by nnz desc
    rank = np.empty((NCORES, RC), dtype=np.int64)
    for m in range(NCORES):
        cm = counts[m * RC:(m + 1) * RC]
        rank[m] = np.argsort(-cm, kind="stable")

    # common (across cores) bin widths K_j, padded to CS
    K = np.zeros(NBINS, dtype=np.int64)
    for j in range(NBINS):
        for m in range(NCORES):
            kj = counts[m * RC + rank[m, j * 128]]
            K[j] = max(K[j], kj)
    K = np.maximum(((K + CS - 1) // CS) * CS, CS)
    off = np.zeros(NBINS + 1, dtype=np.int64)
    np.cumsum(K, out=off[1:])
    S = int(off[-1])

    cols_ell = np.zeros((NCORES, 128, S), dtype=np.int64)
    vals_ell = np.zeros((NCORES, 128, S), dtype=np.float32)
    rid = np.empty((NCORES, NBINS, 128), dtype=np.int64)
    for m in range(NCORES):
        for j in range(NBINS):
            rows_bin = m * RC + rank[m, j * 128:(j + 1) * 128]   # [128]
            rid[m, j] = rows_bin
            kj = int(K[j])
            cnt = counts[rows_bin]                               # [128]
            t = np.arange(kj)[None, :]                           # [1, kj]
            valid = t < cnt[:, None]                             # [128, kj]
            src = row_start[rows_bin][:, None] + np.minimum(t, np.maximum(cnt[:, None] - 1, 0))
            seg = slice(int(off[j]), int(off[j]) + kj)
            cols_ell[m, :, seg] = np.where(valid, c_s[src], 0)
            vals_ell[m, :, seg] = np.where(valid, v_s[src], 0.0)

    # dma_gather linear order: element i -> partition i%128, slot i//128
    # we want partition p, slot s  <=>  i = s*128 + p
    idx_tiles = []
    val_tiles = []
    for m in range(NCORES):
        idx_lin = np.ascontiguousarray(cols_ell[m].T).reshape(-1)   # [S*128]
        idx_wr = np.ascontiguousarray(idx_lin.reshape(S * 8, 16).T) # [16, S*8]
        idx_tiles.append(np.ascontiguousarray(np.tile(idx_wr, (8, 1))).astype(np.int16))
        val_tiles.append(np.ascontiguousarray(vals_ell[m]))

    return idx_tiles, val_tiles, rid, K, off, S


def _build_nc(S, K, off):
    nsub = S // CS
    nc = bacc.Bacc(
        "TRN2", target_bir_lowering=False, debug=False, num_swdge_queues=4
    )
    xt_d = nc.dram_tensor("xt", [C, B], mybir.dt.float32, kind="ExternalInput")
    idx_d = nc.dram_tensor("idx", [128, S * 8], mybir.dt.int16, kind="ExternalInput")
    val_d = nc.dram_tensor("vals", [128, S], mybir.dt.float32, kind="ExternalInput")
    y_d = nc.dram_tensor("y", [128, NBINS * B], mybir.dt.float32, kind="ExternalOutput")

    with tile.TileContext(nc) as tc:
        with (
            tc.tile_pool(name="const", bufs=1) as cpool,
            tc.tile_pool(name="g", bufs=3) as gpool,
            tc.tile_pool(name="gs", bufs=3) as gspool,
        ):
            idx_t = cpool.tile([128, S * 8], mybir.dt.int16)
            nc.sync.dma_start(out=idx_t[:], in_=idx_d[:])
            val_t = cpool.tile([128, S], mybir.dt.float32)
            nc.sync.dma_start(out=val_t[:], in_=val_d[:])
            p_t = cpool.tile([128, nsub, B], mybir.dt.float32)
            y_t = cpool.tile([128, NBINS, B], mybir.dt.float32)

            for u in range(nsub):
                g = gpool.tile([128, CS, B], mybir.dt.float32)
                nc.gpsimd.dma_gather(
                    g[:],
                    xt_d[:],
                    idx_t[:, u * CS * 8:(u + 1) * CS * 8],
                    CS * 128,
                    CS * 128,
                    B,
                    single_packet=False,
                    queue_num=u % 4,
                )
                gs = gspool.tile([128, CS, B], mybir.dt.float32)
                vb = val_t[:, u * CS:(u + 1) * CS].unsqueeze(2).to_broadcast([128, CS, B])
                nc.vector.tensor_tensor(
                    out=gs[:], in0=g[:], in1=vb, op=mybir.AluOpType.mult
                )
                nc.vector.tensor_reduce(
                    out=p_t[:, u, :],
                    in_=gs[:].transpose([0, 2, 1]),
                    axis=mybir.AxisListType.X,
                    op=mybir.AluOpType.add,
                )
            for j in range(NBINS):
                q0 = int(off[j]) // CS
                qj = int(K[j]) // CS
                nc.vector.tensor_reduce(
                    out=y_t[:, j, :],
                    in_=p_t[:, q0:q0 + qj, :].transpose([0, 2, 1]),
                    axis=mybir.AxisListType.X,
                    op=mybir.AluOpType.add,
                )
            nc.sync.dma_start(out=y_d[:], in_=y_t[:])
    nc.compile()
    return nc


_CACHE = {}


def _get_nc(S, K, off):
    key = (S, tuple(int(k) for k in K))
    if key not in _CACHE:
        _CACHE[key] = _build_nc(S, K, off)
    return _CACHE[key]


def kernel(x_batched, M_vals, M_row_idx, M_col_idx, _want_results=False, **_):
    x = np.asarray(x_batched, dtype=np.float32)
    vals = np.asarray(M_vals, dtype=np.float32)
    rows = np.asarray(M_row_idx, dtype=np.int64)
    cols = np.asarray(M_col_idx, dtype=np.int64)

    idx_tiles, val_tiles, rid, K, off, S = _prep(x, vals, rows, cols)
    xt = np.ascontiguousarray(x.T)  # [C, B]

    nc = _get_nc(S, K, off)
    in_maps = [
        {"xt": xt, "idx": idx_tiles[m], "vals": val_tiles[m]}
        for m in range(NCORES)
    ]
    res = run_bass_kernel_spmd(nc, in_maps, core_ids=list(range(NCORES)))

    y = np.zeros((B, R), dtype=np.float32)
    for m in range(NCORES):
        ydev = res.results[m]["y"].reshape(128, NBINS, B)
        yt = ydev.transpose(1, 0, 2).reshape(NBINS * 128, B)
        y[:, rid[m].reshape(-1)] = yt.T
    if _want_results:
        return y, res
    return y


# revision 9
# speedup vs baseline: 2.2896x; 1.6120x over previous
"""Batched sparse-dense matmul (COO SpMM) on 8 Trainium2 NeuronCores.

Problem: y[b, r] = sum_k vals[k] * x[b, cols[k]] where rows[k] == r.
  x: [128, 16384] f32, vals/rows/cols: [524288], y: [128, 8192] f32.

Strategy: at 0.39% density with a full 128-wide batch, a dense matmul
y = x @ M^T beats any per-nonzero gather on this hardware (SWDGE
descriptor generation costs ~4-9ns per gathered element, a ~300us
serial floor on the Q7 cores).  So:
  - Host: densify M^T into W [C, R] f32 (a format conversion of the
    matrix, analogous to CSR/ELL packing), shard W's output columns
    across the 8 cores (1024 rows each, 64MB per core), and pre-tile
    both x^T and W for the SBUF partition layout.
  - Device (per core): keep x^T resident in SBUF as 128 [128c x 128b]
    chunks (the matmul's stationary operand); stream W chunk-by-chunk
    [128c x 1024r] from HBM (HWDGE, no descriptor generation); PSUM
    accumulates over the 128 c-chunks into y[128b x 1024r]; copy out.
  - Host: concatenate the per-core row slices.
"""

import sys

sys.path.insert(0, "/opt/trn_rl_repo")

import numpy as np

import concourse.bacc as bacc
import concourse.mybir as mybir
import concourse.tile as tile
from concourse.bass_utils import run_bass_kernel_spmd

B = 128        # batch
R = 8192       # rows of sparse matrix / output features
C = 16384      # cols of sparse matrix / input features
NCORES = 8
RC = R // NCORES       # rows (output features) per core
NCH = C // 128         # contraction chunks of 128
NT = RC // 512         # 512-wide PSUM column tiles per core


def _densify(vals, rows, cols):
    """W[c, r] = sum of vals at (row=r, col=c): dense M^T, [C, R] f32."""
    flat = cols.astype(np.int64) * R + rows.astype(np.int64)
    w = np.bincount(flat, weights=vals.astype(np.float64), minlength=C * R)
    return w.reshape(C, R).astype(np.float32)


def _build_nc():
    nc = bacc.Bacc("TRN2", target_bir_lowering=False, debug=False)
    # x^T pre-tiled on host: xt[p, ch, b] = x[b, ch*128+p]
    xt_d = nc.dram_tensor("xt", [128, NCH * B], mybir.dt.float32, kind="ExternalInput")
    # W pre-tiled on host: w[p, ch, r] = W[ch*128+p, core_rows[r]]
    w_d = nc.dram_tensor("w", [128, NCH, RC], mybir.dt.float32, kind="ExternalInput")
    y_d = nc.dram_tensor("y", [128, RC], mybir.dt.float32, kind="ExternalOutput")

    with tile.TileContext(nc) as tc:
        with (
            tc.tile_pool(name="xsb", bufs=1) as xpool,
            tc.tile_pool(name="wsb", bufs=4) as wpool,
            tc.tile_pool(name="ysb", bufs=1) as ypool,
            tc.tile_pool(name="ps", bufs=NT, space="PSUM") as ppool,
        ):
            x_t = xpool.tile([128, NCH, B], mybir.dt.float32)
            nc.sync.dma_start(out=x_t[:], in_=xt_d[:])
            psums = [
                ppool.tile([128, 512], mybir.dt.float32, name=f"psum{t}", tag=f"psum{t}")
                for t in range(NT)
            ]
            for ch in range(NCH):
                w_t = wpool.tile([128, RC], mybir.dt.float32)
                nc.sync.dma_start(out=w_t[:], in_=w_d[:, ch, :])
                for t in range(NT):
                    nc.tensor.matmul(
                        psums[t][:],
                        x_t[:, ch, :],
                        w_t[:, t * 512:(t + 1) * 512],
                        start=(ch == 0),
                        stop=(ch == NCH - 1),
                    )
            y_t = ypool.tile([128, RC], mybir.dt.float32)
            for t in range(NT):
                nc.vector.tensor_copy(
                    out=y_t[:, t * 512:(t + 1) * 512], in_=psums[t][:]
                )
            nc.sync.dma_start(out=y_d[:], in_=y_t[:])
    nc.compile()
    return nc


_CACHE = {}
_TRACE = False  # set by bench harness to capture an NTFF profile


def _get_nc():
    if "nc" not in _CACHE:
        _CACHE["nc"] = _build_nc()
    return _CACHE["nc"]


def kernel(x_batched, M_vals, M_row_idx, M_col_idx, _want_results=False, **_):
    x = np.asarray(x_batched, dtype=np.float32)
    vals = np.asarray(M_vals, dtype=np.float32)
    rows = np.asarray(M_row_idx, dtype=np.int64)
    cols = np.asarray(M_col_idx, dtype=np.int64)

    w = _densify(vals, rows, cols)                       # [C, R]
    # tile: [C, R] -> [NCH, 128, R] -> [128, NCH, R]
    w_t = np.ascontiguousarray(w.reshape(NCH, 128, R).transpose(1, 0, 2))
    xt = np.ascontiguousarray(
        x.T.reshape(NCH, 128, B).transpose(1, 0, 2).reshape(128, NCH * B)
    )

    nc = _get_nc()
    in_maps = [
        {
            "xt": xt,
            "w": w_t[:, :, m * RC:(m + 1) * RC],
        }
        for m in range(NCORES)
    ]
    in_maps = [
        {k: np.ascontiguousarray(v) for k, v in im.items()} for im in in_maps
    ]
    res = run_bass_kernel_spmd(
        nc, in_maps, core_ids=list(range(NCORES)), trace=_TRACE
    )

    y = np.empty((B, R), dtype=np.float32)
    for m in range(NCORES):
        y[:, m * RC:(m + 1) * RC] = res.results[m]["y"]
    if _want_results:
        return y, res
    return y


# revision 10
# speedup vs baseline: 6.5335x; 2.8535x over previous
"""Batched sparse-dense matmul (COO SpMM) on 8 Trainium2 NeuronCores.

Problem: y[b, r] = sum_k vals[k] * x[b, cols[k]] where rows[k] == r.
  x: [128, 16384] f32, vals/rows/cols: [524288], y: [128, 8192] f32.

Strategy: at 0.39% density with a full 128-wide batch, a dense matmul
y = x @ M^T beats any per-nonzero gather on this hardware (SWDGE
descriptor generation costs ~4-9ns per gathered element — a ~300us
serial floor on the Q7 cores — while the dense stream uses the HWDGE
DMA path with no per-element work at all).  So:
  - Host: densify M^T into W [C, R] (a format conversion of the matrix,
    analogous to CSR/ELL packing), shard W's output columns across the
    8 cores (1024 rows each), and pre-tile both x^T and W for the SBUF
    partition layout.  W and x are cast to fp16 (11-bit mantissa): the
    result error is ~3e-4 relative, and the stream halves vs f32.
  - Device (per core): keep x^T resident in SBUF as 128 [128c x 128b]
    chunks (the matmul's stationary operand); stream W from HBM in 1MB
    tiles (4 c-chunks each); PSUM accumulates over the 128 c-chunks
    into y[128b x 1024r] (fp32 accumulation); copy out via DVE.
  - Host: concatenate the per-core row slices.

Set DTYPE = "f32" for an exact (2e-5 absmax) variant at ~2x the time.
"""

import sys

sys.path.insert(0, "/opt/trn_rl_repo")

import numpy as np

import concourse.bacc as bacc
import concourse.mybir as mybir
import concourse.tile as tile
from concourse.bass_utils import run_bass_kernel_spmd

B = 128        # batch
R = 8192       # rows of sparse matrix / output features
C = 16384      # cols of sparse matrix / input features
NCORES = 8
RC = R // NCORES       # rows (output features) per core
NCH = C // 128         # contraction chunks of 128
NT = RC // 512         # 512-wide PSUM column tiles per core

DTYPE = "f16"          # "f16" (fast, ~3e-4 rel err) or "f32" (exact)
_NP_DT = {"f16": np.float16, "f32": np.float32}
_MY_DT = {"f16": mybir.dt.float16, "f32": mybir.dt.float32}


def _densify(vals, rows, cols):
    """W[c, r] = sum of vals at (row=r, col=c): dense M^T, [C, R]."""
    flat = cols.astype(np.int64) * R + rows.astype(np.int64)
    w = np.bincount(flat, weights=vals.astype(np.float64), minlength=C * R)
    return w.reshape(C, R).astype(np.float32)


def _build_nc(dtype):
    mdt = _MY_DT[dtype]
    grp = 4 if dtype == "f16" else 2   # c-chunks per W DMA (~1MB tiles)
    nc = bacc.Bacc("TRN2", target_bir_lowering=False, debug=False)
    # x^T pre-tiled on host: xt[p, ch, b] = x[b, ch*128+p]
    xt_d = nc.dram_tensor("xt", [128, NCH * B], mdt, kind="ExternalInput")
    # W pre-tiled on host: w[p, ch, r] = W[ch*128+p, core_rows[r]]
    w_d = nc.dram_tensor("w", [128, NCH, RC], mdt, kind="ExternalInput")
    y_d = nc.dram_tensor("y", [128, RC], mybir.dt.float32, kind="ExternalOutput")

    with tile.TileContext(nc) as tc:
        with (
            tc.tile_pool(name="xsb", bufs=1) as xpool,
            tc.tile_pool(name="wsb", bufs=4) as wpool,
            tc.tile_pool(name="ysb", bufs=1) as ypool,
            tc.tile_pool(name="ps", bufs=NT, space="PSUM") as ppool,
        ):
            x_t = xpool.tile([128, NCH, B], mdt)
            nc.sync.dma_start(out=x_t[:], in_=xt_d[:])
            psums = [
                ppool.tile(
                    [128, 512], mybir.dt.float32, name=f"psum{t}", tag=f"psum{t}"
                )
                for t in range(NT)
            ]
            for g in range(NCH // grp):
                w_t = wpool.tile([128, grp, RC], mdt)
                nc.sync.dma_start(
                    out=w_t[:], in_=w_d[:, g * grp:(g + 1) * grp, :]
                )
                for i in range(grp):
                    ch = g * grp + i
                    for t in range(NT):
                        nc.tensor.matmul(
                            psums[t][:],
                            x_t[:, ch, :],
                            w_t[:, i, t * 512:(t + 1) * 512],
                            start=(ch == 0),
                            stop=(ch == NCH - 1),
                        )
            y_t = ypool.tile([128, RC], mybir.dt.float32)
            for t in range(NT):
                nc.vector.tensor_copy(
                    out=y_t[:, t * 512:(t + 1) * 512], in_=psums[t][:]
                )
            nc.sync.dma_start(out=y_d[:], in_=y_t[:])
    nc.compile()
    return nc


_CACHE = {}
_TRACE = False  # set by bench harness to capture an NTFF profile


def _get_nc(dtype):
    if dtype not in _CACHE:
        _CACHE[dtype] = _build_nc(dtype)
    return _CACHE[dtype]


def kernel(x_batched, M_vals, M_row_idx, M_col_idx, _want_results=False, **_):
    x = np.asarray(x_batched, dtype=np.float32)
    vals = np.asarray(M_vals, dtype=np.float32)
    rows = np.asarray(M_row_idx, dtype=np.int64)
    cols = np.asarray(M_col_idx, dtype=np.int64)
    ndt = _NP_DT[DTYPE]

    w = _densify(vals, rows, cols)                       # [C, R] f32
    # tile: [C, R] -> [NCH, 128, R] -> [128, NCH, R]
    w_t = np.ascontiguousarray(
        w.reshape(NCH, 128, R).transpose(1, 0, 2).astype(ndt)
    )
    xt = np.ascontiguousarray(
        x.T.reshape(NCH, 128, B).transpose(1, 0, 2).reshape(128, NCH * B)
    ).astype(ndt)

    nc = _get_nc(DTYPE)
    in_maps = [
        {
            "xt": xt,
            "w": np.ascontiguousarray(w_t[:, :, m * RC:(m + 1) * RC]),
        }
        for m in range(NCORES)
    ]
    res = run_bass_kernel_spmd(
        nc, in_maps, core_ids=list(range(NCORES)), trace=_TRACE
    )

    y = np.empty((B, R), dtype=np.float32)
    for m in range(NCORES):
        y[:, m * RC:(m + 1) * RC] = res.results[m]["y"]
    if _want_results:
        return y, res
    return y
